# revision 2
# baseline (speedup 1.0000x reference)
"""RNN-T joint network kernel for 8 Trainium2 NeuronCores — fp8 DoubleRow version.

Reference computation:
    enc_proj = enc_out @ W_enc.T + b_enc          # [B,T,J]
    pred_proj = pred_out @ W_dec.T + b_dec        # [B,U,J]
    joint    = tanh(e[:,:,None,:] + d[:,None,:,:])
    out      = joint @ W_out.T + b_out            # [B,T,U,V]

Shapes (hardcoded): B=4, T=256, U=128, D=512, J=640, V=1024.
Sharding: data-parallel over B*T rows; core k handles batch k//2, t-range
(k%2)*128..+128. Each core emits its [128,128,1024] f32 output slab.

Algorithm (per core) — ANOVA residual decomposition so the dominant
[T*U, J] @ [J, V] GEMM can run in fp8-e4m3 DoubleRow mode (2 contraction
rows per PE cell per cycle) while staying within the accuracy budget:

    tanh(e+d) = f(e) + g(d) + rho(e,d),   f(y)=KE*tanh(LE*y), g(y)=KD*tanh(LD*y)
    out[t,u,v] = sum_j rho*W[v,j]  (fp8 DR GEMM, rms(rho)~0.11)
               + FW[t,v] + GW[u,v] + b_out[v]     (exact compensation)

  - FW = f(e) @ W.T and GW = g(d) @ W.T are tiny bf16 GEMMs.
  - GW + b_out is added by DVE on the PSUM->SBUF pass (partition-aligned).
  - FW[t,:] rides the third DR matmul's spare pair slot: stationary slot1
    holds (1.0 @p0, 0.0625 @p1, zeros), moving slot1 holds fp8 hi/lo rows
    of FW (lo pre-scaled by 16), giving FW to ~2^-8 accuracy for free.
  - The residual lattice is built by ACT (tanh w/ per-partition e-bias) and
    one fused DVE scalar_tensor_tensor per chunk: (jt - Fe[:,t]) - Gd -> fp8.
"""

import os
import numpy as np

B, T, U, D, J, V = 4, 256, 128, 512, 640, 1024
NCORES = 8
TC = (B * T) // NCORES          # 128 t-rows per core
JC = J // 128                   # 5 j-chunks
DC = D // 128                   # 4 d-chunks
G = 8                           # t-rows per FW-row group
NG = TC // G                    # 16 groups
NB_LAT = 4                      # lattice pool depth

# f/g fitted to the smoothed-tanh ANOVA optimum for this input distribution.
KE, LE = 1.0445730270745623, 0.7123092507565199
KD, LD = 1.0440554505780621, 0.7153731220309161

MAIN_DT_NAME = "float8e4_dr"

_CACHE = {}


def _build_bass():
    import concourse.mybir as mybir
    import concourse.tile as tile
    import concourse.bacc as bacc

    f32 = mybir.dt.float32
    bf16 = mybir.dt.bfloat16
    fp8 = mybir.dt.float8e4
    DR = mybir.MatmulPerfMode.DoubleRow
    Tanh = mybir.ActivationFunctionType.Tanh
    Sub = mybir.AluOpType.subtract

    nc = bacc.Bacc("TRN2", debug=False)

    enc_d = nc.dram_tensor("enct", [D, TC], bf16, kind="ExternalInput")
    pred_d = nc.dram_tensor("predt", [D, U], bf16, kind="ExternalInput")
    wenc_d = nc.dram_tensor("wenct", [D, J], bf16, kind="ExternalInput")
    wdec_d = nc.dram_tensor("wdect", [D, J], bf16, kind="ExternalInput")
    wo8a_d = nc.dram_tensor("wo8a", [128, 2, V], fp8, kind="ExternalInput")
    wo8b_d = nc.dram_tensor("wo8b", [128, 2, V], fp8, kind="ExternalInput")
    mstat_d = nc.dram_tensor("mstat", [128, 2, G, V], fp8, kind="ExternalInput")
    latc_d = nc.dram_tensor("latc", [128, U], fp8, kind="ExternalInput")
    wo16_d = nc.dram_tensor("wo16", [128, JC, V], bf16, kind="ExternalInput")
    benc_d = nc.dram_tensor("bencr", [128, JC], f32, kind="ExternalInput")
    bdec_d = nc.dram_tensor("bdecr", [128, JC], f32, kind="ExternalInput")
    bout_d = nc.dram_tensor("boutr", [128, V], f32, kind="ExternalInput")
    out_d = nc.dram_tensor("out", [TC, U, V], f32, kind="ExternalOutput")

    enc_ap, pred_ap = enc_d.ap(), pred_d.ap()
    wenc_ap, wdec_ap = wenc_d.ap(), wdec_d.ap()
    out_ap = out_d.ap()

    with tile.TileContext(nc) as tc:
        with (
            tc.tile_pool(name="consts", bufs=1) as consts,
            tc.tile_pool(name="proj", bufs=1) as proj,
            tc.tile_pool(name="jtp", bufs=3) as jtp,
            tc.tile_pool(name="latp", bufs=NB_LAT) as latp,
            tc.tile_pool(name="osb", bufs=5) as osbp,
            tc.tile_pool(name="psB", bufs=4, space="PSUM") as psB,
        ):
            # ---- load inputs; projection operands first so PE can start ----
            enc_t, pred_t, wenc_t, wdec_t = [], [], [], []
            for dc in range(DC):
                sl = slice(dc * 128, (dc + 1) * 128)
                a = consts.tile([128, TC], bf16, tag=f"enc{dc}")
                nc.sync.dma_start(a[:], enc_ap[sl, :])
                enc_t.append(a)
                p = consts.tile([128, U], bf16, tag=f"pred{dc}")
                nc.sync.dma_start(p[:], pred_ap[sl, :])
                pred_t.append(p)
                we = consts.tile([128, J], bf16, tag=f"wenc{dc}")
                nc.sync.dma_start(we[:], wenc_ap[sl, :])
                wenc_t.append(we)
                wd = consts.tile([128, J], bf16, tag=f"wdec{dc}")
                nc.sync.dma_start(wd[:], wdec_ap[sl, :])
                wdec_t.append(wd)

            benc_t = consts.tile([128, JC], f32, tag="benc")
            nc.sync.dma_start(benc_t[:], benc_d.ap()[:])
            bdec_t = consts.tile([128, JC], f32, tag="bdec")
            nc.sync.dma_start(bdec_t[:], bdec_d.ap()[:])
            wo16_t = consts.tile([128, JC, V], bf16, tag="wo16")
            nc.sync.dma_start(wo16_t[:], wo16_d.ap()[:])
            wo8a_t = consts.tile([128, 2, V], fp8, tag="wo8a")
            nc.sync.dma_start(wo8a_t[:], wo8a_d.ap()[:])
            wo8b_t = consts.tile([128, 2, V], fp8, tag="wo8b")
            nc.sync.dma_start(wo8b_t[:], wo8b_d.ap()[:])
            bout_t = consts.tile([128, V], f32, tag="bout")
            nc.sync.dma_start(bout_t[:], bout_d.ap()[:])
            m_t = []
            for mb in range(2):
                m = consts.tile([128, 2, G, V], fp8, tag=f"m{mb}")
                nc.sync.dma_start(m[:], mstat_d.ap()[:])
                m_t.append(m)

            # ---- projections: encP[c][j,t] f32, decP[c][j,u] bf16 ----
            encP, decP = [], []
            for c in range(JC):
                jsl = slice(c * 128, (c + 1) * 128)
                pse = psB.tile([128, TC], f32, tag="ps")
                for dc in range(DC):
                    nc.tensor.matmul(pse[:], wenc_t[dc][:, jsl], enc_t[dc][:],
                                     start=(dc == 0), stop=(dc == DC - 1))
                e = proj.tile([128, TC], f32, tag=f"encP{c}")
                nc.vector.tensor_scalar_add(e[:], pse[:], benc_t[:, c:c + 1])
                encP.append(e)

                psd = psB.tile([128, U], f32, tag="ps")
                for dc in range(DC):
                    nc.tensor.matmul(psd[:], wdec_t[dc][:, jsl], pred_t[dc][:],
                                     start=(dc == 0), stop=(dc == DC - 1))
                d = proj.tile([128, U], bf16, tag=f"decP{c}")
                nc.vector.tensor_scalar_add(d[:], psd[:], bdec_t[:, c:c + 1])
                decP.append(d)

            # ---- f(e), g(d): kappa*tanh(lambda*y), bf16 (used identically
            # in the lattice subtraction and the compensation GEMMs) ----
            fe16 = proj.tile([128, JC, TC], bf16, tag="fe16")
            gd16 = proj.tile([128, JC * U], bf16, tag="gd16")
            for c in range(JC):
                ft = proj.tile([128, TC], bf16, tag="ftmp", bufs=2)
                nc.scalar.activation(ft[:], encP[c][:], Tanh, scale=LE)
                nc.vector.tensor_scalar_mul(fe16[:, c, :], ft[:], KE)
                gt = proj.tile([128, U], bf16, tag="gtmp", bufs=2)
                nc.scalar.activation(gt[:], decP[c][:], Tanh, scale=LD)
                nc.vector.tensor_scalar_mul(gd16[:, c * U:(c + 1) * U], gt[:], KD)
            fe32 = proj.tile([128, JC, TC], f32, tag="fe32")
            nc.vector.tensor_copy(fe32[:], fe16[:])

            # ---- FW = f(e) @ W.T (bf16), then fp8 hi/lo rows ----
            psf = psB.tile([128, V], f32, tag="ps")
            for vh in range(2):
                vsl = slice(vh * 512, (vh + 1) * 512)
                for c in range(JC):
                    nc.tensor.matmul(psf[:, vsl], fe16[:, c, :], wo16_t[:, c, vsl],
                                     start=(c == 0), stop=(c == JC - 1))
            fw32 = proj.tile([128, V], f32, tag="fw32")
            nc.vector.tensor_copy(fw32[:], psf[:])
            fwhi = proj.tile([128, V], fp8, tag="fwhi")
            nc.vector.tensor_copy(fwhi[:], fw32[:])
            fwrem = proj.tile([128, V], f32, tag="fwrem")
            nc.vector.tensor_sub(fwrem[:], fw32[:], fwhi[:])
            fwlo = proj.tile([128, V], fp8, tag="fwlo")
            nc.vector.tensor_scalar_mul(fwlo[:], fwrem[:], 16.0)

            # ---- GWb = g(d) @ W.T + b_out (f32, partition-aligned w/ out) ----
            psg = psB.tile([128, V], f32, tag="ps")
            for vh in range(2):
                vsl = slice(vh * 512, (vh + 1) * 512)
                for c in range(JC):
                    nc.tensor.matmul(psg[:, vsl], gd16[:, c * U:(c + 1) * U],
                                     wo16_t[:, c, vsl],
                                     start=(c == 0), stop=(c == JC - 1))
            gwb = proj.tile([128, V], f32, tag="gwb")
            nc.vector.tensor_add(gwb[:], psg[:], bout_t[:])

            # ---- pre-init lattice pool buffers: slot 5 holds the constant
            # stationary weights for the FW pair (1.0@p0, 1/16@p1, 0 rest) ----
            lat_init = []
            for _ in range(NB_LAT):
                lt = latp.tile([128, 6, U], fp8, tag="lat")
                nc.sync.dma_start(lt[:, 5, :], latc_d.ap()[:])
                lat_init.append(lt)

            # FW hi/lo rows for group 0 into m buffer 0
            nc.sync.dma_start(m_t[0][0:1, 1, :, :], fwhi[0:G, :])
            nc.sync.dma_start(m_t[0][1:2, 1, :, :], fwlo[0:G, :])

            # ---- main loop ----
            for g in range(NG):
                if g + 1 < NG:
                    mnext = m_t[(g + 1) % 2]
                    rs = slice((g + 1) * G, (g + 2) * G)
                    nc.sync.dma_start(mnext[0:1, 1, :, :], fwhi[rs, :])
                    nc.sync.dma_start(mnext[1:2, 1, :, :], fwlo[rs, :])
                mcur = m_t[g % 2]
                for i in range(G):
                    t = g * G + i
                    jt = jtp.tile([128, JC * U], bf16, tag="jt")
                    for c in range(JC):
                        nc.scalar.activation(jt[:, c * U:(c + 1) * U], decP[c][:],
                                             Tanh, bias=encP[c][:, t:t + 1])
                    lat = latp.tile([128, 6, U], fp8, tag="lat")
                    for c in range(JC):
                        nc.vector.scalar_tensor_tensor(
                            lat[:, c, :], jt[:, c * U:(c + 1) * U],
                            fe32[:, c, t:t + 1], gd16[:, c * U:(c + 1) * U],
                            Sub, Sub)

                    ps = psB.tile([128, V], f32, tag="ps")
                    for c3 in range(3):
                        lhs = lat[:, 2 * c3:2 * c3 + 2, :]
                        for vh in range(2):
                            vsl = slice(vh * 512, (vh + 1) * 512)
                            rhs = (wo8a_t[:, :, vsl] if c3 == 0 else
                                   wo8b_t[:, :, vsl] if c3 == 1 else
                                   mcur[:, :, i, vsl])
                            nc.tensor.matmul(ps[:, vsl], lhs, rhs,
                                             start=(c3 == 0), stop=(c3 == 2),
                                             perf_mode=DR)

                    osb = osbp.tile([128, V], f32, tag="osb")
                    if g == NG - 1 and i >= G - 2:
                        for vh in range(2):
                            vsl = slice(vh * 512, (vh + 1) * 512)
                            nc.vector.tensor_add(osb[:, vsl], ps[:, vsl],
                                                 gwb[:, vsl])
                            nc.sync.dma_start(out_ap[t][:, vsl], osb[:, vsl])
                    else:
                        nc.vector.tensor_add(osb[:], ps[:], gwb[:])
                        nc.sync.dma_start(out_ap[t], osb[:])

    nc.compile()
    return nc


def _host_prep(enc_out, pred_out, W_enc, b_enc, W_dec, b_dec, W_out, b_out):
    import concourse.mybir as mybir
    np_bf16 = np.dtype(mybir.dt.np(mybir.dt.bfloat16))
    np_fp8 = np.dtype(mybir.dt.np(mybir.dt.float8e4))

    wencT = np.ascontiguousarray(np.asarray(W_enc, np.float32).T).astype(np_bf16)
    wdecT = np.ascontiguousarray(np.asarray(W_dec, np.float32).T).astype(np_bf16)
    woT = np.ascontiguousarray(np.asarray(W_out, np.float32).T)      # [J, V]

    wo16 = np.ascontiguousarray(
        woT.reshape(JC, 128, V).transpose(1, 0, 2)).astype(np_bf16)  # [128,JC,V]
    wo8 = woT.astype(np_fp8)                                         # [J, V]
    wo8a = np.ascontiguousarray(
        wo8[0:256].reshape(2, 128, V).transpose(1, 0, 2))            # [128,2,V]
    wo8b = np.ascontiguousarray(
        wo8[256:512].reshape(2, 128, V).transpose(1, 0, 2))
    mstat = np.zeros((128, 2, G, V), np_fp8)
    mstat[:, 0, :, :] = wo8[512:640][:, None, :]                     # W c4 x G
    latc = np.zeros((128, U), np_fp8)
    latc[0, :] = 1.0
    latc[1, :] = 0.0625

    bencr = np.ascontiguousarray(
        np.asarray(b_enc, np.float32).reshape(JC, 128).T)
    bdecr = np.ascontiguousarray(
        np.asarray(b_dec, np.float32).reshape(JC, 128).T)
    boutr = np.ascontiguousarray(
        np.broadcast_to(np.asarray(b_out, np.float32), (128, V)))

    in_maps = []
    for k in range(NCORES):
        b, th = k // 2, (k % 2) * TC
        encT = np.ascontiguousarray(
            np.asarray(enc_out[b, th:th + TC], np.float32).T).astype(np_bf16)
        predT = np.ascontiguousarray(
            np.asarray(pred_out[b], np.float32).T).astype(np_bf16)
        in_maps.append({
            "enct": encT, "predt": predT, "wenct": wencT, "wdect": wdecT,
            "wo8a": wo8a, "wo8b": wo8b, "mstat": mstat, "latc": latc,
            "wo16": wo16, "bencr": bencr, "bdecr": bdecr, "boutr": boutr,
        })
    return in_maps


def kernel(enc_out, pred_out, W_enc, b_enc, W_dec, b_dec, W_out, b_out):
    from concourse import bass_utils

    if "nc" not in _CACHE:
        _CACHE["nc"] = _build_bass()
    nc = _CACHE["nc"]

    in_maps = _host_prep(enc_out, pred_out, W_enc, b_enc, W_dec, b_dec,
                         W_out, b_out)

    trace = bool(int(os.environ.get("TRNK_PROFILE", "0")))
    res = bass_utils.run_bass_kernel_spmd(
        nc, in_maps, core_ids=list(range(NCORES)), trace=trace)
    kernel.last_exec_ns = res.exec_time_ns

    full = np.empty((B, T, U, V), np.float32)
    for k in range(NCORES):
        b, th = k // 2, (k % 2) * TC
        full[b, th:th + TC] = res.results[k]["out"]
    return full


kernel.last_exec_ns = None
